# revision 26
# baseline (speedup 1.0000x reference)
"""ChebConv (K=3, two layers) GNN kernel for 8 Trainium2 NeuronCores.

Strategy (graph/data parallel, per sharding hint):
  - Nodes are partitioned into 8 contiguous ranges (12500 per core); each core
    owns the scatter-sum output for its dst range.
  - Edges are bucketed by dst block (128 dst nodes per block) on the host;
    per block, edges are packed into chunks of 128 (padded).
  - One propagate (h -> segment_sum(norm * h[src], dst)) per dst block:
      * SWDGE dma_gather of the (pre-scaled, bf16) source rows, one
        instruction per (block, node-quadrant) since indices are int16:
        G[p, c*TF:(c+1)*TF] = table[gidx[chunk c, lane p]]
      * selection matrices S[e, d] = (slot[e] == d) built with a single
        broadcast is_equal on the vector engine
      * TensorE matmuls S^T @ G accumulate the per-dst-slot sums in PSUM
  - The symmetric normalization -dinv[src]*dinv[dst] is folded in by storing
    gather tables pre-scaled by dinv (u = dinv * h) and scaling the PSUM
    result by -dinv[dst] on evacuation.
  - After each propagate whose result other cores need, an AllGather
    replicates the scaled table (bf16) to every core.
  - Dense parts (x @ W_k, bias, relu, Chebyshev recurrence) are fused into
    per-block epilogues: PE transposes the node-major tile, then matmuls with
    the replicated weights; bias is added via a rank-1 (K=1) matmul.
"""

import math
import os

import numpy as np

P = 128


def _ceil_div(a, b):
    return (a + b - 1) // b


def build_program(cfg, x, edge_index, W1, b1, W2, b2):
    import concourse.bacc as bacc
    import concourse.tile as tile
    from concourse import bass, mybir
    from concourse.bass_utils import run_bass_kernel_spmd
    from concourse.masks import make_identity

    f32 = mybir.dt.float32
    bf16 = mybir.dt.bfloat16
    i32 = mybir.dt.int32
    bf16_np = mybir.dt.np(bf16)
    AF = mybir.ActivationFunctionType
    OP = mybir.AluOpType

    N = cfg["N"]
    E = cfg["E"]
    IN = cfg["IN"]
    HID = cfg["HID"]
    OUT = cfg["OUT"]
    ncores = cfg["ncores"]
    n_loc = N // ncores
    assert n_loc * ncores == N
    nb = _ceil_div(n_loc, P)
    nbP = nb * P

    x = np.asarray(x, dtype=np.float32)
    src = np.asarray(edge_index[0]).astype(np.int64)
    dst = np.asarray(edge_index[1]).astype(np.int64)
    W1 = np.asarray(W1, dtype=np.float32)
    b1 = np.asarray(b1, dtype=np.float32)
    W2 = np.asarray(W2, dtype=np.float32)
    b2 = np.asarray(b2, dtype=np.float32)

    # ---- host-side graph preprocessing (sharding prep) ----
    deg = np.bincount(src, minlength=N).astype(np.float32)
    dinv = np.where(deg > 0, 1.0 / np.sqrt(np.maximum(deg, 1.0)), 0.0).astype(
        np.float32
    )

    # Gather tables are indexed with int16 (dma_gather), so split node space
    # into NSPLIT sub-tables; edges are bucketed by (core, dst block, split).
    NSPLIT = 4
    rows_split = _ceil_div(N, NSPLIT)
    assert rows_split <= 32000

    qsplit = src // rows_split
    lidx16 = (src - qsplit * rows_split).astype(np.int16)

    core_all = dst // n_loc
    loc = dst - core_all * n_loc
    blk_all = loc // P
    slot_all = (loc - blk_all * P).astype(np.float32)

    key = (core_all * nb + blk_all) * NSPLIT + qsplit
    order = np.argsort(key, kind="stable")
    key = key[order]
    s_lidx = lidx16[order]
    s_slot = slot_all[order]
    core_of = core_all[order]
    blk = blk_all[order]
    qs = qsplit[order]

    counts = np.bincount(key, minlength=ncores * nb * NSPLIT).reshape(
        ncores, nb, NSPLIT
    )
    # chunks per (block, split): shared across cores (SPMD program)
    CQ = _ceil_div(counts.max(axis=0), P)  # [nb, NSPLIT]
    cqcum = np.zeros((nb, NSPLIT + 1), dtype=np.int64)
    np.cumsum(CQ, axis=1, out=cqcum[:, 1:])
    Ctot = cqcum[:, NSPLIT]  # chunks per block
    Cmax = int(Ctot.max())

    # Gather instructions are merged over GROUP consecutive dst blocks (one
    # dma_gather per (group, split)) to amortize the ~1us SWDGE fixed cost.
    # Chunks stay block-pure; within a group the gather's chunk order is
    # (split-major, then block, then chunk).
    # Merge gathers across GROUP dst blocks to amortize the ~1us SWDGE fixed
    # cost per dma_gather. Gathers over 1024 indices exceed the 64-descriptor
    # per-engine packet ceiling and must use single_packet=False.
    GROUP = cfg.get("group", 4)
    ngrp = _ceil_div(nb, GROUP)
    grp_of = np.arange(nb) // GROUP
    CQ_grp = np.zeros((ngrp, NSPLIT), dtype=np.int64)
    np.add.at(CQ_grp, grp_of, CQ)
    qofs = np.zeros((ngrp, NSPLIT + 1), dtype=np.int64)
    np.cumsum(CQ_grp, axis=1, out=qofs[:, 1:])
    Gchunks = qofs[:, NSPLIT]
    Gchmax = int(Gchunks.max())
    GICmax = 8 * Gchmax
    # per-block prefix of chunks within its (group, split)
    pref = np.zeros_like(CQ)
    for g in range(ngrp):
        bs = slice(g * GROUP, min((g + 1) * GROUP, nb))
        pref[bs] = np.cumsum(CQ[bs], axis=0) - CQ[bs]
    # chunk offset of (block, split) within the group's G tile
    bofs = qofs[grp_of, :NSPLIT] + pref  # [nb, NSPLIT]

    starts = np.zeros(ncores * nb * NSPLIT, dtype=np.int64)
    cnt_flat = counts.reshape(-1)
    np.cumsum(cnt_flat[:-1], out=starts[1:])
    j = np.arange(E, dtype=np.int64) - starts[key]
    chunk_g = cqcum[blk, qs] + j // P  # chunk id within block (S order)
    lane = j % P

    # slots+dinv metadata: col 0 = dinv, cols 1.. = per-chunk slot values
    slotv = np.full((ncores, nb, P, Cmax + 1), 300.0, dtype=np.float32)
    slotv[core_of, blk, lane, 1 + chunk_g] = s_slot

    # int16 gather indices in 16-partition wrap, replicated to 128 partitions,
    # laid out per group in gather order
    gidx16 = np.zeros((ncores, ngrp, 16, GICmax), dtype=np.int16)
    L = pref[blk, qs] * P + j  # logical position within the (group, split) gather
    col = 8 * qofs[grp_of[blk], qs] + L // 16
    gidx16[core_of, grp_of[blk], L % 16, col] = s_lidx
    gidx = np.tile(gidx16, (1, 1, 8, 1))  # [ncores, ngrp, 128, GICmax]

    tmp = dinv.reshape(ncores, n_loc)
    pad = np.zeros((ncores, nbP - n_loc), dtype=np.float32)
    dinv_blocks = np.concatenate([tmp, pad], axis=1).reshape(ncores, nb, P)
    slotv[:, :, :, 0] = dinv_blocks
    meta = slotv

    xpad = np.concatenate(
        [x.reshape(ncores, n_loc, IN), np.zeros((ncores, nbP - n_loc, IN), np.float32)],
        axis=1,
    )
    u0 = (dinv[:, None] * x).astype(bf16_np)  # pre-scaled gather table, bf16
    TF = 128  # gather-table row width (L2 tables padded to 128 cols, 256B rows)
    i16 = mybir.dt.int16

    # ---- build the SPMD program ----
    from concourse import library_config

    nc = bacc.Bacc(
        "TRN2",
        target_bir_lowering=False,
        debug=False,
        num_devices=ncores,
        num_swdge_queues=4,
    )

    u0_d = nc.dram_tensor("u0", [N, IN], bf16, kind="ExternalInput").ap()
    xloc_d = nc.dram_tensor("xloc", [nbP, IN], f32, kind="ExternalInput").ap()
    gidx_d = nc.dram_tensor("gidx", [ngrp, P, GICmax], i16, kind="ExternalInput").ap()
    meta_d = nc.dram_tensor("meta", [nb, P, Cmax + 1], f32, kind="ExternalInput").ap()
    w1_d = nc.dram_tensor("w1", [3, IN, HID], f32, kind="ExternalInput").ap()
    b1_d = nc.dram_tensor("b1", [HID], f32, kind="ExternalInput").ap()
    w2_d = nc.dram_tensor("w2", [3, HID, OUT], f32, kind="ExternalInput").ap()
    b2_d = nc.dram_tensor("b2", [OUT], f32, kind="ExternalInput").ap()
    out_d = nc.dram_tensor("out", [n_loc, OUT], f32, kind="ExternalOutput").ap()

    groups = [list(range(ncores))]

    from contextlib import ExitStack

    with ExitStack() as ctx:
        tc = ctx.enter_context(tile.TileContext(nc))

        dram = ctx.enter_context(tc.tile_pool(name="dram", bufs=1, space="DRAM"))
        # AllGather outputs in the Shared scratchpad (fast HBM-HBM collective
        # path); .opt()-compatible APs via .ap()
        u1_full = nc.dram_tensor("u1_full", [N, IN], bf16, addr_space="Shared")
        uh_full = nc.dram_tensor("uh_full", [N, TF], bf16, addr_space="Shared")
        ut1_full = nc.dram_tensor("ut1_full", [N, TF], bf16, addr_space="Shared")
        u1_loc = dram.tile([n_loc, IN], bf16, tag="u1_loc")
        uh_loc = dram.tile([n_loc, TF], bf16, tag="uh_loc")
        ut1_loc = dram.tile([n_loc, TF], bf16, tag="ut1_loc")
        tx1_loc = dram.tile([nbP, IN], f32, tag="tx1_loc")
        h_loc = dram.tile([nbP, HID], f32, tag="h_loc")
        th1_loc = dram.tile([nbP, HID], f32, tag="th1_loc")

        const = ctx.enter_context(tc.tile_pool(name="const", bufs=1))
        io = ctx.enter_context(tc.tile_pool(name="io", bufs=6))
        gp = ctx.enter_context(tc.tile_pool(name="gp", bufs=3))
        sp = ctx.enter_context(tc.tile_pool(name="sp", bufs=6))
        ev = ctx.enter_context(tc.tile_pool(name="ev", bufs=3))
        pps = ctx.enter_context(tc.tile_pool(name="pps", bufs=2, space="PSUM"))
        tps = ctx.enter_context(tc.tile_pool(name="tps", bufs=2, space="PSUM"))
        dps = ctx.enter_context(tc.tile_pool(name="dps", bufs=2, space="PSUM"))

        ident = const.tile([P, P], f32, tag="ident")
        make_identity(nc, ident[:])
        iota_i = const.tile([P, P], i32, tag="iota_i")
        nc.gpsimd.iota(iota_i[:], pattern=[[1, P]], base=0, channel_multiplier=0)
        iota_f = const.tile([P, P], f32, tag="iota_f")
        nc.vector.tensor_copy(iota_f[:], iota_i[:])
        nc.gpsimd.load_library(library_config.mlp)

        w1_t = []
        for k in range(3):
            t = const.tile([IN, HID], f32, tag=f"w1_{k}")
            nc.sync.dma_start(t[:], w1_d[k])
            w1_t.append(t)
        w2_t = []
        for k in range(3):
            t = const.tile([HID, OUT], f32, tag=f"w2_{k}")
            nc.sync.dma_start(t[:], w2_d[k])
            w2_t.append(t)
        ones1 = const.tile([1, P], f32, tag="ones1")
        nc.vector.memset(ones1[:], 1.0)
        b1_t = const.tile([1, HID], f32, tag="b1_t")
        nc.sync.dma_start(b1_t[:], b1_d[None, :])
        b2_t = const.tile([1, OUT], f32, tag="b2_t")
        nc.sync.dma_start(b2_t[:], b2_d[None, :])

        def propagate(table_ap, F, epilogue):
            # table_ap: [N, TF] bf16; F = feature cols actually used (<= TF)
            for g in range(ngrp):
                off_t = io.tile([P, GICmax], i16, tag="off")
                nc.sync.dma_start(off_t[:], gidx_d[g])
                G = gp.tile([P, Gchmax * TF], bf16, tag="G")
                for q in range(NSPLIT):
                    cgq = int(CQ_grp[g, q])
                    if cgq == 0:
                        continue
                    c0 = int(qofs[g, q])
                    n_q = cgq * P
                    r0 = q * rows_split
                    r1 = min(r0 + rows_split, N)
                    nc.gpsimd.dma_gather(
                        G[:, c0 * TF : (c0 + cgq) * TF].rearrange(
                            "p (c f) -> p c f", f=TF
                        ),
                        table_ap[r0:r1],
                        off_t[:, 8 * c0 : 8 * (c0 + cgq)],
                        n_q,
                        n_q,
                        TF,
                        queue_num=q,
                        single_packet=(n_q <= 1024),
                    )
                for b in range(g * GROUP, min((g + 1) * GROUP, nb)):
                    CT = int(Ctot[b])
                    meta_t = io.tile([P, Cmax + 1], f32, tag="meta")
                    nc.sync.dma_start(meta_t[:], meta_d[b])
                    S = sp.tile([P, Cmax * P], bf16, tag="S")
                    nc.vector.tensor_tensor(
                        out=S[:, : CT * P].rearrange("p (c q) -> p c q", q=P),
                        in0=meta_t[:, 1 : 1 + CT].to_broadcast([P, CT, P]),
                        in1=iota_f[:, None, :].to_broadcast([P, CT, P]),
                        op=OP.is_equal,
                    )
                    ps = pps.tile([P, F], f32, tag="prop_ps")
                    if CT == 0:
                        nc.vector.memset(ps[:], 0.0)
                    for q in range(NSPLIT):
                        for c in range(int(CQ[b, q])):
                            kk = int(cqcum[b, q]) + c  # S chunk index
                            gg = int(bofs[b, q]) + c  # G chunk index (group)
                            nc.tensor.matmul(
                                out=ps[:],
                                lhsT=S[:, kk * P : (kk + 1) * P],
                                rhs=G[:, gg * TF : gg * TF + F],
                                start=(kk == 0),
                                stop=(kk == CT - 1),
                            )
                    epilogue(b, ps, meta_t[:, 0:1])

        def rows_of(b):
            return min(P, n_loc - b * P)

        # ---- layer 1, propagate #1: Tx1 = -Ds A Ds x ----
        def epi1(b, ps, dinv_ap):
            rows = rows_of(b)
            tx1 = ev.tile([P, IN], f32, tag="tx1")
            nc.vector.tensor_scalar(
                out=tx1[:], in0=ps[:], scalar1=dinv_ap, scalar2=-1.0,
                op0=OP.mult, op1=OP.mult,
            )
            u1 = ev.tile([P, IN], bf16, tag="u1")
            nc.vector.tensor_scalar(
                out=u1[:], in0=tx1[:], scalar1=dinv_ap, scalar2=None, op0=OP.mult
            )
            nc.sync.dma_start(tx1_loc[:][b * P : b * P + P], tx1[:])
            nc.sync.dma_start(u1_loc[:][b * P : b * P + rows], u1[:rows])

        propagate(u0_d, IN, epi1)
        nc.gpsimd.collective_compute(
            "AllGather", OP.bypass, replica_groups=groups,
            ins=[u1_loc.opt()], outs=[u1_full.ap()],
        )

        # ---- layer 1, propagate #2 + dense layer 1 ----
        def epi2(b, ps, dinv_ap):
            rows = rows_of(b)
            tx2 = ev.tile([P, IN], f32, tag="tx2")
            nc.vector.tensor_scalar(
                out=tx2[:], in0=ps[:], scalar1=dinv_ap, scalar2=-2.0,
                op0=OP.mult, op1=OP.mult,
            )
            x_t = ev.tile([P, IN], f32, tag="x_t")
            nc.sync.dma_start(x_t[:], xloc_d[b * P : (b + 1) * P])
            nc.vector.tensor_tensor(out=tx2[:], in0=tx2[:], in1=x_t[:], op=OP.subtract)
            tx1 = ev.tile([P, IN], f32, tag="tx1b")
            nc.sync.dma_start(tx1[:], tx1_loc[:][b * P : (b + 1) * P])
            outps = dps.tile([P, HID], f32, tag="dps")
            for k, t in enumerate([x_t, tx1, tx2]):
                tp = tps.tile([IN, P], f32, tag="trp")
                nc.tensor.transpose(tp[:], t[:], ident[:])
                tsb = ev.tile([IN, P], f32, tag="trs")
                nc.scalar.activation(tsb[:], tp[:], AF.Copy)
                nc.tensor.matmul(
                    out=outps[:], lhsT=tsb[:], rhs=w1_t[k][:],
                    start=(k == 0), stop=False, skip_group_check=True,
                )
            nc.tensor.matmul(
                out=outps[:], lhsT=ones1[:1, :], rhs=b1_t[:1, :],
                start=False, stop=True, skip_group_check=True,
            )
            h_t = ev.tile([P, HID], f32, tag="h_t")
            nc.scalar.activation(h_t[:], outps[:], AF.Relu)
            uh = ev.tile([P, TF], bf16, tag="uh")
            nc.vector.memset(uh[:, HID:TF], 0.0)
            nc.vector.tensor_scalar(
                out=uh[:, 0:HID], in0=h_t[:], scalar1=dinv_ap, scalar2=None,
                op0=OP.mult,
            )
            nc.sync.dma_start(h_loc[:][b * P : b * P + P], h_t[:])
            nc.sync.dma_start(uh_loc[:][b * P : b * P + rows], uh[:rows])

        propagate(u1_full.ap(), IN, epi2)
        nc.gpsimd.collective_compute(
            "AllGather", OP.bypass, replica_groups=groups,
            ins=[uh_loc.opt()], outs=[uh_full.ap()],
        )

        # ---- layer 2, propagate #1: Th1 ----
        def epi3(b, ps, dinv_ap):
            rows = rows_of(b)
            th1 = ev.tile([P, HID], f32, tag="th1")
            nc.vector.tensor_scalar(
                out=th1[:], in0=ps[:], scalar1=dinv_ap, scalar2=-1.0,
                op0=OP.mult, op1=OP.mult,
            )
            ut1 = ev.tile([P, TF], bf16, tag="ut1")
            nc.vector.memset(ut1[:, HID:TF], 0.0)
            nc.vector.tensor_scalar(
                out=ut1[:, 0:HID], in0=th1[:], scalar1=dinv_ap, scalar2=None,
                op0=OP.mult,
            )
            nc.sync.dma_start(th1_loc[:][b * P : b * P + P], th1[:])
            nc.sync.dma_start(ut1_loc[:][b * P : b * P + rows], ut1[:rows])

        propagate(uh_full.ap(), HID, epi3)
        nc.gpsimd.collective_compute(
            "AllGather", OP.bypass, replica_groups=groups,
            ins=[ut1_loc.opt()], outs=[ut1_full.ap()],
        )

        # ---- layer 2, propagate #2 + dense layer 2 + output ----
        def epi4(b, ps, dinv_ap):
            rows = rows_of(b)
            th2 = ev.tile([P, HID], f32, tag="th2")
            nc.vector.tensor_scalar(
                out=th2[:], in0=ps[:], scalar1=dinv_ap, scalar2=-2.0,
                op0=OP.mult, op1=OP.mult,
            )
            h_t = ev.tile([P, HID], f32, tag="h_t2")
            nc.sync.dma_start(h_t[:], h_loc[:][b * P : (b + 1) * P])
            nc.vector.tensor_tensor(out=th2[:], in0=th2[:], in1=h_t[:], op=OP.subtract)
            th1 = ev.tile([P, HID], f32, tag="th1b")
            nc.sync.dma_start(th1[:], th1_loc[:][b * P : (b + 1) * P])
            outps = dps.tile([P, OUT], f32, tag="dps")
            for k, t in enumerate([h_t, th1, th2]):
                tp = tps.tile([HID, P], f32, tag="trp")
                nc.tensor.transpose(tp[:], t[:], ident[:])
                tsb = ev.tile([HID, P], f32, tag="trs2")
                nc.scalar.activation(tsb[:], tp[:], AF.Copy)
                nc.tensor.matmul(
                    out=outps[:], lhsT=tsb[:], rhs=w2_t[k][:],
                    start=(k == 0), stop=False, skip_group_check=True,
                )
            nc.tensor.matmul(
                out=outps[:], lhsT=ones1[:1, :], rhs=b2_t[:1, :],
                start=False, stop=True, skip_group_check=True,
            )
            o_t = ev.tile([P, OUT], f32, tag="o_t")
            nc.scalar.activation(o_t[:], outps[:], AF.Copy)
            nc.sync.dma_start(out_d[b * P : b * P + rows], o_t[:rows])

        propagate(ut1_full.ap(), HID, epi4)

    nc.compile()

    in_map = lambda c: {
        "u0": u0,
        "xloc": np.ascontiguousarray(xpad[c]),
        "gidx": np.ascontiguousarray(gidx[c]),
        "meta": np.ascontiguousarray(meta[c]),
        "w1": W1,
        "b1": b1,
        "w2": W2,
        "b2": b2,
    }
    in_maps = [in_map(c) for c in range(ncores)]
    return nc, in_maps


def build_and_run(cfg, x, edge_index, W1, b1, W2, b2, trace=False):
    from concourse.bass_utils import run_bass_kernel_spmd

    ncores = cfg["ncores"]
    nc, in_maps = build_program(cfg, x, edge_index, W1, b1, W2, b2)
    res = run_bass_kernel_spmd(nc, in_maps, list(range(ncores)), trace=trace)
    out = np.concatenate([res.results[c]["out"] for c in range(ncores)], axis=0)
    return out, res


def kernel(x, edge_index, W1, b1, W2, b2):
    cfg = dict(N=100000, E=1600000, IN=128, HID=64, OUT=40, ncores=8)
    trace = os.environ.get("CHEB_TRACE", "0") == "1"
    out, res = build_and_run(cfg, x, edge_index, W1, b1, W2, b2, trace=trace)
    if trace and res.exec_time_ns is not None:
        print(f"HW exec time: {res.exec_time_ns} ns")
    return out
